# revision 1
# baseline (speedup 1.0000x reference)
"""FSUMGU cell on 8 Trainium2 NeuronCores.

Math (per reference):
    zf = [hx, x] @ w_f.T + b_f
    fg = (zf + 1) / 2
    fgx = fg * hx
    ng = [fgx, x] @ w_n.T + b_n
    hy = (1 - fg) * ng + fgx

Sharding: 2 batch-halves (r) x 4 hidden-quarters (c); core id = r*4 + c.
Each core computes hy[r-half, c-quarter]. The only cross-core dependency
is ng's contraction over the full hidden dim of fgx, satisfied with one
AllGather of bf16 fgx^T over each 4-core row group.

On-core: activations/weights are PE-transposed into [k, *] bf16 SBUF
tiles so every matmul is out[b,h] += catT[k,b].T @ wT[k,h] with fp32
PSUM accumulation. fp32->bf16 happens inside SWDGE cast-DMAs. PE
transposes are interleaved into the matmul stream in small groups so
the tensor engine never idles long enough for HAM to re-throttle the
clock. Phase 2 accumulates its input-half contraction first so those
matmuls (plus the w_n transposes) hide the AllGather latency.
"""
import sys

sys.path.insert(0, "/opt/trn_rl_repo")

import numpy as np
import concourse.bass as bass
import concourse.tile as tile
from concourse import bacc, mybir, masks
from concourse.bass_utils import run_bass_kernel_spmd

F32 = mybir.dt.float32
BF16 = mybir.dt.bfloat16

B, H, I = 2048, 2048, 2048
R, C = 2, 4
BL = B // R            # 1024 rows of batch per core
HC = H // C            # 512 output features per core
NB = BL // 128         # 8 batch tiles
NKH = H // 128         # 16 k-tiles in the hx / fgx part
NKI = I // 128         # 16 k-tiles in the input part
NK = NKH + NKI         # 32 k-tiles total contraction
NHT = HC // 128        # 4 h-tiles per core slice

_NC_CACHE = None


def build():
    nc = bacc.Bacc(None, target_bir_lowering=False, debug=False)
    d_inp = nc.dram_tensor("inp", [BL, I], F32, kind="ExternalInput").ap()
    d_hx = nc.dram_tensor("hx", [BL, H], F32, kind="ExternalInput").ap()
    d_hxc = nc.dram_tensor("hxc", [BL, HC], F32, kind="ExternalInput").ap()
    d_wf = nc.dram_tensor("wf", [HC, H + I], F32, kind="ExternalInput").ap()
    d_wn = nc.dram_tensor("wn", [HC, H + I], F32, kind="ExternalInput").ap()
    d_bf = nc.dram_tensor("bf", [1, HC], F32, kind="ExternalInput").ap()
    d_bn = nc.dram_tensor("bn", [1, HC], F32, kind="ExternalInput").ap()
    d_hy = nc.dram_tensor("hy", [BL, HC], F32, kind="ExternalOutput").ap()

    with tile.TileContext(nc) as tc:
        with (
            tc.tile_pool(name="const", bufs=1) as const,
            tc.tile_pool(name="wT", bufs=1) as wT_pool,          # wfT then wnT (time-shared)
            tc.tile_pool(name="big", bufs=1) as big_pool,        # hxT then gather (time-shared) + inputT
            tc.tile_pool(name="persist", bufs=1) as persist,
            tc.tile_pool(name="aload", bufs=4) as aload,
            tc.tile_pool(name="wload", bufs=2) as wload,
            tc.tile_pool(name="wnload", bufs=2) as wnload,
            tc.tile_pool(name="scr", bufs=2) as scr,
            tc.tile_pool(name="fgtr", bufs=1) as fgtr,
            tc.tile_pool(name="outp", bufs=2) as outp,
            tc.tile_pool(name="dram", bufs=1, space="DRAM") as dram,
            tc.tile_pool(name="ps_acc", bufs=5, space="PSUM") as ps_acc,
            tc.tile_pool(name="ps_tp", bufs=3, space="PSUM") as ps_tp,
        ):
            ident = const.tile([128, 128], BF16, tag="ident")
            masks.make_identity(nc, ident[:])
            ones = const.tile([1, 128], BF16, tag="ones")
            nc.vector.memset(ones[:], 1.0)

            # ---- persistent transposed tensors
            hxT = big_pool.tile([128, NKH, BL], BF16, tag="big_hx")      # hx^T  (phase 1)
            inputT = big_pool.tile([128, NKI, BL], BF16, tag="big_inp")  # input^T (both phases)
            wfT = wT_pool.tile([128, NK, HC], BF16, tag="wT")
            fg_hxT = persist.tile([128, NHT, BL], BF16, tag="fghxT")
            omfgN = persist.tile([128, NB, HC], BF16, tag="omfg")
            fghxN = persist.tile([128, NB, HC], BF16, tag="fghx")
            hxcN = persist.tile([128, NB, HC], BF16, tag="hxc")
            spillN = persist.tile([128, 5, HC], BF16, tag="spill")

            # DRAM bounce buffers for the two collectives (split 3/5 so the
            # first gather can trigger early, mid phase 1)
            NB1 = 3
            HB = NB1 * 128
            HB2 = BL - HB
            cc_in1 = dram.tile([HC, HB], BF16)
            cc_in2 = dram.tile([HC, HB2], BF16)
            cc_out1 = dram.tile([C, HC, HB], BF16)
            cc_out2 = dram.tile([C, HC, HB2], BF16)

            eng_state = [0]

            def emit_tp_group(src_b16, src_k0, n_kt, dst, dst_ti0, dst_col):
                """PE-transpose n_kt (<=4) [128,128] slices + one batched copy."""
                tp = ps_tp.tile([128, 512], BF16, tag="tp")
                for j in range(n_kt):
                    nc.tensor.matmul(
                        tp[:, j * 128:(j + 1) * 128],
                        src_b16[:, (src_k0 + j) * 128:(src_k0 + j + 1) * 128],
                        ident[:],
                        is_transpose=True,
                    )
                dst_ap = dst[:, dst_ti0:dst_ti0 + n_kt, dst_col:dst_col + 128]
                src_ap = tp[:, :n_kt * 128].rearrange("p (a f) -> p a f", f=128)
                if eng_state[0] % 2 == 0:
                    nc.vector.tensor_copy(dst_ap, src_ap)
                else:
                    nc.scalar.copy(dst_ap, src_ap)
                eng_state[0] += 1

            filler = []   # queued (tag, fn) transpose groups, drained between MM bursts

            def drain(n):
                for _ in range(min(n, len(filler))):
                    filler.pop(0)[1]()

            def drain_until(tag):
                """Emit every queued group with tag <= `tag` (correctness gate)."""
                while filler and filler[0][0] <= tag:
                    filler.pop(0)[1]()

            def queue_act_tiles(bt):
                """Load + queue transposes for b-tile bt's activations."""
                bs = bt * 128
                achunk = aload.tile([128, NKH * 128], BF16, tag="aload")
                nc.gpsimd.dma_start(achunk[:], d_hx[bs:bs + 128, :])
                ichunk = aload.tile([128, NKI * 128], BF16, tag="aload")
                nc.gpsimd.dma_start(ichunk[:], d_inp[bs:bs + 128, :])
                nc.gpsimd.dma_start(hxcN[:, bt, :], d_hxc[bs:bs + 128, :])
                for g in range(0, NKH, 4):
                    filler.append((bt, lambda g=g, t=achunk: emit_tp_group(t, g, 4, hxT, g, bs)))
                for g in range(0, NKI, 4):
                    filler.append((bt, lambda g=g, t=ichunk: emit_tp_group(t, g, 4, inputT, g, bs)))

            # ---- w_f k-half 0 + first activations: minimal deps for first matmul
            def load_wf_half(kh, dst):
                for a in range(NHT):
                    wchunk = wload.tile([128, NKH * 128], BF16, tag="wload")
                    nc.gpsimd.dma_start(
                        wchunk[:], d_wf[a * 128:(a + 1) * 128, kh * 2048:(kh + 1) * 2048])
                    for g in range(0, NKH, 4):
                        emit_tp_group(wchunk, g, 4, dst, kh * NKH + g, a * 128)

            load_wf_half(0, wfT)
            queue_act_tiles(0)
            queue_act_tiles(1)
            drain_until(1)

            # ---- bias prep: bfp=(b_f+1)/2, bfm=1-bfp, bn; broadcast to 128 partitions
            bf_row = const.tile([1, HC], F32, tag="bfrow")
            bn_row = const.tile([1, HC], F32, tag="bnrow")
            nc.sync.dma_start(bf_row[:], d_bf[:])
            nc.sync.dma_start(bn_row[:], d_bn[:])
            bfp_row = const.tile([1, HC], F32, tag="bfprow")
            bfm_row = const.tile([1, HC], F32, tag="bfmrow")
            nc.vector.tensor_scalar(bfp_row[:], bf_row[:], 0.5, 0.5,
                                    mybir.AluOpType.mult, mybir.AluOpType.add)
            nc.vector.tensor_scalar(bfm_row[:], bfp_row[:], -1.0, 1.0,
                                    mybir.AluOpType.mult, mybir.AluOpType.add)
            bias_bc = const.tile([128, 3, HC], BF16, tag="biasbc")
            for bi, row in enumerate((bfp_row, bfm_row, bn_row)):
                row16 = const.tile([1, HC], BF16, tag=f"row16_{bi}")
                nc.vector.tensor_copy(row16[:], row[:])
                pb = ps_tp.tile([128, HC], F32, tag="tp")
                nc.tensor.matmul(pb[:], ones[:], row16[:], start=True, stop=True)
                nc.vector.tensor_copy(bias_bc[:, bi, :], pb[:])
            bfp_bc = bias_bc[:, 0, :]
            bfm_bc = bias_bc[:, 1, :]
            bn_bc = bias_bc[:, 2, :]

            wn_anchor = [None]

            # ---- phase 1 per batch tile: dense MM stream + interleaved fillers
            for bt in range(NB):
                bs = bt * 128
                if bt + 2 < NB:
                    queue_act_tiles(bt + 2)
                drain_until(bt)  # this tile's operands must be emitted already
                acc = ps_acc.tile([128, HC], F32, tag="acc")
                for j in range(NK):
                    # interleave queued transposes only once DMA runs ahead of
                    # the PE; early on a not-yet-loaded group would stall the
                    # statically-ordered matmul stream.
                    if bt >= 3 and j % 8 == 4:
                        drain(1)
                    if bt == 0 and j == NKH:
                        # w_f k-half 1 transposes slot in after bt0's first
                        # half-contraction, hiding their DMA behind real work
                        load_wf_half(1, wfT)
                    lhsT = (hxT[:, j, bs:bs + 128] if j < NKH
                            else inputT[:, j - NKH, bs:bs + 128])
                    nc.tensor.matmul(
                        acc[:], lhsT, wfT[:, j, :],
                        start=(j == 0), stop=(j == NK - 1),
                    )
                # fg = 0.5*acc + bfp ; omfg = bfm - 0.5*acc ; fgx = fg * hxc
                fg_t = fgtr.tile([128, HC], BF16, tag="fg")
                fg_inst = nc.vector.scalar_tensor_tensor(
                    fg_t[:], acc[:], 0.5, bfp_bc,
                    mybir.AluOpType.mult, mybir.AluOpType.add)
                if bt == 2:
                    wn_anchor[0] = fg_inst
                nc.vector.scalar_tensor_tensor(
                    omfgN[:, bt, :], acc[:], -0.5, bfm_bc,
                    mybir.AluOpType.mult, mybir.AluOpType.add)
                nc.vector.tensor_mul(fghxN[:, bt, :], fg_t[:], hxcN[:, bt, :])
                # transpose fgx tile -> fg_hxT[:, :, bs:bs+128] (small, HAM-safe)
                tp = ps_tp.tile([128, 512], BF16, tag="tp")
                for a in range(NHT):
                    nc.tensor.matmul(
                        tp[:, a * 128:(a + 1) * 128],
                        fghxN[:, bt, a * 128:(a + 1) * 128],
                        ident[:],
                        is_transpose=True,
                    )
                nc.scalar.copy(
                    fg_hxT[:, :, bs:bs + 128],
                    tp[:].rearrange("p (a f) -> p a f", f=128),
                )
                # stream this b-tile's fgx^T columns to the collective input
                cc_in_half = cc_in1 if bt < NB1 else cc_in2
                hb = bs if bt < NB1 else bs - HB
                nc.sync.dma_start(
                    cc_in_half.rearrange("(a p) b -> p a b", p=128)[:, :, hb:hb + 128],
                    fg_hxT[:, :, bs:bs + 128])
                if bt == NB1 - 1:
                    # first-half all-gather rides under the rest of phase 1
                    nc.gpsimd.collective_compute(
                        "AllGather",
                        mybir.AluOpType.bypass,
                        replica_groups=[[0, 1, 2, 3], [4, 5, 6, 7]],
                        ins=[cc_in1.opt()],
                        outs=[cc_out1.opt()],
                    )

            # ---- second-half all-gather
            nc.gpsimd.collective_compute(
                "AllGather",
                mybir.AluOpType.bypass,
                replica_groups=[[0, 1, 2, 3], [4, 5, 6, 7]],
                ins=[cc_in2.opt()],
                outs=[cc_out2.opt()],
            )

            # ---- w_n: load input-half (k-tiles 16..31) first, transpose all.
            # This dense block (plus phase-2's input-half matmuls) runs during
            # the AllGather, so PE idle time there is already covered.
            wnT = wT_pool.tile([128, NK, HC], BF16, tag="wT")
            from concourse.tile import add_dep_helper
            for kh in (1, 0):
                for a in range(NHT):
                    wchunk = wnload.tile([128, NKH * 128], BF16, tag="wnload")
                    wdma = nc.gpsimd.dma_start(
                        wchunk[:], d_wn[a * 128:(a + 1) * 128, kh * 2048:(kh + 1) * 2048])
                    # keep w_n traffic out of phase 1's DMA window
                    add_dep_helper(wdma.ins, wn_anchor[0].ins, sync=True,
                                   reason="defer w_n loads past mid phase 1")
                    for g in range(0, NKH, 4):
                        emit_tp_group(wchunk, g, 4, wnT, kh * NKH + g, a * 128)
            drain(len(filler))  # flush any remaining queued act groups
            assert not filler

            # ---- read back gathered fgx^T (reuses hxT's slot)
            gat = big_pool.tile([128, NKH, BL], BF16, tag="big_hx")
            for j in range(NKH):
                q, rr = j // NHT, (j % NHT) * 128
                nc.sync.dma_start(gat[:, j, :HB], cc_out1[q, rr:rr + 128, :])
                nc.sync.dma_start(gat[:, j, HB:], cc_out2[q, rr:rr + 128, :])

            # ---- phase 2. B-tiles 0-4 hold their PSUM bank across both
            # contraction halves; 5-7 run their (CC-independent) input half
            # eagerly and spill the partial to SBUF so those matmuls fit in
            # the second all-gather's window.
            def epilogue(bt, acc, spill=None):
                bs = bt * 128
                t = scr.tile([128, HC], F32, tag="t")
                nc.vector.tensor_add(t[:], acc[:], bn_bc)
                if spill is not None:
                    nc.vector.tensor_add(t[:], t[:], spill)
                u = scr.tile([128, HC], F32, tag="u")
                nc.vector.tensor_mul(u[:], omfgN[:, bt, :], t[:])
                o = outp.tile([128, HC], F32, tag="o")
                nc.vector.tensor_add(o[:], u[:], fghxN[:, bt, :])
                nc.sync.dma_start(d_hy[bs:bs + 128, :], o[:])

            for bt in range(NB1):
                bs = bt * 128
                acc = ps_acc.tile([128, HC], F32, tag="acc")
                korder = list(range(NKH, NK)) + list(range(NKH))
                for idx, j in enumerate(korder):
                    lhsT = (gat[:, j, bs:bs + 128] if j < NKH
                            else inputT[:, j - NKH, bs:bs + 128])
                    nc.tensor.matmul(
                        acc[:], lhsT, wnT[:, j, :],
                        start=(idx == 0), stop=(idx == NK - 1),
                    )
                epilogue(bt, acc)
            for bt in range(NB1, NB):
                bs = bt * 128
                acc = ps_acc.tile([128, HC], F32, tag="acc")
                for idx, j in enumerate(range(NKH, NK)):
                    nc.tensor.matmul(
                        acc[:], inputT[:, j - NKH, bs:bs + 128], wnT[:, j, :],
                        start=(idx == 0), stop=(idx == NKH - 1),
                    )
                nc.vector.tensor_copy(spillN[:, bt - NB1, :], acc[:])
            for bt in range(NB1, NB):
                bs = bt * 128
                acc = ps_acc.tile([128, HC], F32, tag="acc")
                for idx, j in enumerate(range(NKH)):
                    nc.tensor.matmul(
                        acc[:], gat[:, j, bs:bs + 128], wnT[:, j, :],
                        start=(idx == 0), stop=(idx == NKH - 1),
                    )
                epilogue(bt, acc, spill=spillN[:, bt - NB1, :])

    nc.finalize()
    return nc


def _get_nc():
    global _NC_CACHE
    if _NC_CACHE is None:
        _NC_CACHE = build()
    return _NC_CACHE


def kernel(input, hx, w_f, b_f, w_n, b_n, **_ignored):
    input = np.ascontiguousarray(np.asarray(input, dtype=np.float32))
    hx = np.ascontiguousarray(np.asarray(hx, dtype=np.float32))
    w_f = np.ascontiguousarray(np.asarray(w_f, dtype=np.float32))
    b_f = np.ascontiguousarray(np.asarray(b_f, dtype=np.float32))
    w_n = np.ascontiguousarray(np.asarray(w_n, dtype=np.float32))
    b_n = np.ascontiguousarray(np.asarray(b_n, dtype=np.float32))

    nc = _get_nc()
    in_maps = []
    for core in range(R * C):
        r, c = core // C, core % C
        in_maps.append({
            "inp": np.ascontiguousarray(input[r * BL:(r + 1) * BL, :]),
            "hx": np.ascontiguousarray(hx[r * BL:(r + 1) * BL, :]),
            "hxc": np.ascontiguousarray(hx[r * BL:(r + 1) * BL, c * HC:(c + 1) * HC]),
            "wf": np.ascontiguousarray(w_f[c * HC:(c + 1) * HC, :]),
            "wn": np.ascontiguousarray(w_n[c * HC:(c + 1) * HC, :]),
            "bf": np.ascontiguousarray(b_f[None, c * HC:(c + 1) * HC]),
            "bn": np.ascontiguousarray(b_n[None, c * HC:(c + 1) * HC]),
        })
    res = run_bass_kernel_spmd(nc, in_maps, list(range(R * C)))
    rows = []
    for r in range(R):
        rows.append(np.concatenate(
            [res.results[r * C + c]["hy"] for c in range(C)], axis=1))
    return np.concatenate(rows, axis=0)


if __name__ == "__main__":
    rng = np.random.default_rng(0)
    inputs = {
        "input": rng.uniform(-1, 1, (B, I)).astype(np.float32),
        "hx": rng.uniform(-1, 1, (B, H)).astype(np.float32),
        "w_f": (rng.standard_normal((H, H + I)) / np.sqrt(H + I)).astype(np.float32),
        "b_f": (rng.standard_normal(H) / np.sqrt(H + I)).astype(np.float32),
        "w_n": (rng.standard_normal((H, H + I)) / np.sqrt(H + I)).astype(np.float32),
        "b_n": (rng.standard_normal(H) / np.sqrt(H + I)).astype(np.float32),
    }
    out = kernel(**inputs)
    x64 = {k: v.astype(np.float64) for k, v in inputs.items()}
    cat = np.concatenate([x64["hx"], x64["input"]], axis=1)
    fg = (cat @ x64["w_f"].T + x64["b_f"] + 1.0) * 0.5
    fgx = fg * x64["hx"]
    ng = np.concatenate([fgx, x64["input"]], axis=1) @ x64["w_n"].T + x64["b_n"]
    exp = (1.0 - fg) * ng + fgx
    err = np.abs(out - exp).max() / np.abs(exp).max()
    print("rel err:", err)



# revision 8
# speedup vs baseline: 1.7109x; 1.7109x over previous
"""FSUMGU cell on 8 Trainium2 NeuronCores — v2.

Math (per reference):
    zf = [hx, x] @ w_f.T + b_f
    fg = (zf + 1) / 2
    fgx = fg * hx
    ng = [fgx, x] @ w_n.T + b_n
    hy = (1 - fg) * ng + fgx

Sharding: 4 batch groups (512 rows) x 2 hidden halves (1024 cols);
core = 2*g + hc.  Pair {2g, 2g+1} shares batch rows, splits hidden.

Formulation: transposed-output [h, b].  Weights are the stationary
matmul operand (lhsT [k,128h]); activations stream as rhs [k, 512b].
All tensors arrive from the host already bf16, transposed, and laid
out partition-major, so the device issues NO PE transposes, NO bias
broadcasts, NO cast traffic: the tensor engine runs only the 512 real
matmuls per core.  Biases are per-partition columns consumed by
scalar-engine Identity activations (out = in*scale + bias).

hx^T k-tiles are ordered my-slice-first (w_f rows reordered to match)
so fgx = fg * hxT[:, a, :] needs no per-core slice of hx.  The fgx
exchange is a pair AllGather done in two chunks (local h-tiles 0-3,
4-7) that ride under GEMM2's input-half contraction; GEMM2 holds all
8 PSUM banks and contracts x-half first, then gathered-fgx halves as
the chunks land.
"""
import sys

sys.path.insert(0, "/opt/trn_rl_repo")

import numpy as np
import ml_dtypes
import concourse.bass as bass
import concourse.tile as tile
from concourse import bacc, mybir
from concourse.bass_utils import run_bass_kernel_spmd

F32 = mybir.dt.float32
BF16 = mybir.dt.bfloat16
BF = ml_dtypes.bfloat16
IDENT = mybir.ActivationFunctionType.Identity

B, H, I = 2048, 2048, 2048
R, C = 4, 2            # batch groups x hidden halves
BL = B // R            # 512 batch rows per core
HC = H // C            # 1024 hidden outputs per core
NHT = HC // 128        # 8 local h-tiles
NKT = 32               # 4096 contraction / 128

_NC_CACHE = None


def build():
    nc = bacc.Bacc(None, target_bir_lowering=False, debug=False)
    # p-major layouts: partition dim first, >=4KB contiguous per partition
    d_hxT = nc.dram_tensor("hxT", [128, 16, BL], BF16, kind="ExternalInput").ap()
    d_inpT = nc.dram_tensor("inpT", [128, 16, BL], BF16, kind="ExternalInput").ap()
    d_wf = nc.dram_tensor("wf", [128, 16, 16, 128], BF16, kind="ExternalInput").ap()
    d_wn = nc.dram_tensor("wn", [128, NHT, NKT, 128], BF16, kind="ExternalInput").ap()
    d_bfp = nc.dram_tensor("bfp", [128, NHT], F32, kind="ExternalInput").ap()
    d_bfm = nc.dram_tensor("bfm", [128, NHT], F32, kind="ExternalInput").ap()
    d_bn = nc.dram_tensor("bn", [128, NHT], F32, kind="ExternalInput").ap()
    d_hy = nc.dram_tensor("hy", [NHT, 128, BL], BF16, kind="ExternalOutput").ap()

    with tile.TileContext(nc) as tc:
        with (
            tc.tile_pool(name="const", bufs=1) as const,
            tc.tile_pool(name="wts", bufs=1) as wts,
            tc.tile_pool(name="acts", bufs=1) as acts,      # hxT (then gat), inpT
            tc.tile_pool(name="persist", bufs=1) as persist,
            tc.tile_pool(name="fg_sc", bufs=2) as fg_sc,
            tc.tile_pool(name="ng_sc", bufs=2) as ng_sc,
            tc.tile_pool(name="hy_sc", bufs=2) as hy_sc,
            tc.tile_pool(name="dram", bufs=1, space="DRAM") as dram,
            tc.tile_pool(name="ps", bufs=8, space="PSUM") as ps,
        ):
            # ---- weight stream (scalar ring): wf chunks first, then wn
            # x-half, then wn fgx-half — ordered by first consumption time.
            wf = wts.tile([128, 16, 16, 128], BF16, tag="wf")
            wn = wts.tile([128, NHT, NKT, 128], BF16, tag="wn")
            for c in range(2):      # chunks for h-tile 0 go first
                nc.scalar.dma_start(wf[:, c], d_wf[:, c])
            bfp = const.tile([128, NHT], F32, tag="bfp")
            bfm = const.tile([128, NHT], F32, tag="bfm")
            bn = const.tile([128, NHT], F32, tag="bn")
            nc.scalar.dma_start(bfp[:], d_bfp[:])
            nc.scalar.dma_start(bfm[:], d_bfm[:])
            nc.scalar.dma_start(bn[:], d_bn[:])
            for c in range(2, 16):
                nc.scalar.dma_start(wf[:, c], d_wf[:, c])
            for a in range(NHT):    # x-half needed first (GEMM2 pass 1)
                nc.scalar.dma_start(wn[:, a, 16:32], d_wn[:, a, 16:32])
            for a in range(NHT):
                nc.scalar.dma_start(wn[:, a, 0:16], d_wn[:, a, 0:16])

            # ---- activation stream (sync ring), 4-kt groups
            hxT = acts.tile([128, 16, BL], BF16, tag="hx_gat")
            inpT = acts.tile([128, 16, BL], BF16, tag="inp")
            for t in range(0, 16, 4):
                nc.sync.dma_start(hxT[:, t:t + 4], d_hxT[:, t:t + 4])
            for t in range(0, 16, 4):
                nc.sync.dma_start(inpT[:, t:t + 4], d_inpT[:, t:t + 4])

            fgx = persist.tile([128, NHT, BL], BF16, tag="fgx")
            omfg = persist.tile([128, NHT, BL], BF16, tag="omfg")

            cc_in1 = dram.tile([128, 4, BL], BF16)
            cc_in2 = dram.tile([128, 4, BL], BF16)
            cc_out1 = dram.tile([2, 128, 4, BL], BF16)
            cc_out2 = dram.tile([2, 128, 4, BL], BF16)

            # ---- GEMM1: zf^T per h-tile; drain to fgx / omfg
            for a in range(NHT):
                acc = ps.tile([128, BL], F32, tag="acc")
                for j in range(NKT):
                    lhsT = wf[:, a * 2 + j // 16, j % 16, :]
                    rhs = hxT[:, j, :] if j < 16 else inpT[:, j - 16, :]
                    nc.tensor.matmul(acc[:], lhsT, rhs,
                                     start=(j == 0), stop=(j == NKT - 1))
                fg_t = fg_sc.tile([128, BL], BF16, tag="fg")
                nc.scalar.activation(fg_t[:], acc[:], IDENT,
                                     bias=bfp[:, a:a + 1], scale=0.5)
                nc.scalar.activation(omfg[:, a, :], acc[:], IDENT,
                                     bias=bfm[:, a:a + 1], scale=-0.5)
                nc.vector.tensor_mul(fgx[:, a, :], fg_t[:], hxT[:, a, :])

            # ---- pair AllGather of fgx^T, two chunks (gpsimd stream:
            # bounce write -> trigger, per chunk; waits ride on gpsimd)
            nc.gpsimd.dma_start(cc_in1[:], fgx[:, 0:4, :])
            nc.gpsimd.collective_compute(
                "AllGather", mybir.AluOpType.bypass,
                replica_groups=[[0, 1], [2, 3], [4, 5], [6, 7]],
                ins=[cc_in1.opt()], outs=[cc_out1.opt()],
            )
            nc.gpsimd.dma_start(cc_in2[:], fgx[:, 4:8, :])
            nc.gpsimd.collective_compute(
                "AllGather", mybir.AluOpType.bypass,
                replica_groups=[[0, 1], [2, 3], [4, 5], [6, 7]],
                ins=[cc_in2.opt()], outs=[cc_out2.opt()],
            )

            # gathered fgx^T in global k order, reusing hxT's SBUF slot
            gat = acts.tile([128, 16, BL], BF16, tag="hx_gat")
            for m in range(2):
                nc.sync.dma_start(gat[:, m * 8:m * 8 + 4], cc_out1[m])
            for m in range(2):
                nc.sync.dma_start(gat[:, m * 8 + 4:m * 8 + 8], cc_out2[m])

            # ---- GEMM2: x-half first (CC-independent), all 8 banks held;
            # then gathered-fgx halves as the chunks land.
            acc2 = []
            for a in range(NHT):
                t = ps.tile([128, BL], F32, tag="acc")
                acc2.append(t)
                for j in range(16, 32):
                    nc.tensor.matmul(t[:], wn[:, a, j, :], inpT[:, j - 16, :],
                                     start=(j == 16), stop=False)
            for a in range(NHT):
                for j in (0, 1, 2, 3, 8, 9, 10, 11):
                    nc.tensor.matmul(acc2[a][:], wn[:, a, j, :], gat[:, j, :],
                                     start=False, stop=False)
            for a in range(NHT):
                for i, j in enumerate((4, 5, 6, 7, 12, 13, 14, 15)):
                    nc.tensor.matmul(acc2[a][:], wn[:, a, j, :], gat[:, j, :],
                                     start=False, stop=(i == 7))
                ng_t = ng_sc.tile([128, BL], F32, tag="ng")
                nc.scalar.activation(ng_t[:], acc2[a][:], IDENT,
                                     bias=bn[:, a:a + 1], scale=1.0)
                hy_t = hy_sc.tile([128, BL], F32, tag="hy")
                nc.vector.tensor_mul(hy_t[:], omfg[:, a, :], ng_t[:])
                nc.vector.tensor_add(hy_t[:], hy_t[:], fgx[:, a, :])
                nc.gpsimd.dma_start(d_hy[a], hy_t[:])  # f32->bf16 cast DMA

    nc.finalize()
    return nc


def _get_nc():
    global _NC_CACHE
    if _NC_CACHE is None:
        _NC_CACHE = build()
    return _NC_CACHE


def make_in_maps(input, hx, w_f, b_f, w_n, b_n):
    """Host-side shard + transpose + bf16 cast into p-major device layouts."""
    input = np.asarray(input, dtype=np.float32)
    hx = np.asarray(hx, dtype=np.float32)
    w_f = np.asarray(w_f, dtype=np.float32)
    b_f = np.asarray(b_f, dtype=np.float32)
    w_n = np.asarray(w_n, dtype=np.float32)
    b_n = np.asarray(b_n, dtype=np.float32)

    in_maps = []
    for core in range(R * C):
        g, hc = core // C, core % C
        rows = slice(g * BL, (g + 1) * BL)
        hsl = slice(hc * HC, (hc + 1) * HC)
        psl = slice((1 - hc) * HC, (2 - hc) * HC)

        # hx^T with k-tiles my-slice-first; [2048,512]->[128p,16kt,512b]
        hxTf = hx[rows].T
        hxTr = np.concatenate([hxTf[hsl], hxTf[psl]], axis=0)
        hxT = np.ascontiguousarray(
            hxTr.reshape(16, 128, BL).transpose(1, 0, 2).astype(BF))
        inpT = np.ascontiguousarray(
            input[rows].T.reshape(16, 128, BL).transpose(1, 0, 2).astype(BF))

        # w_f rows for my h-slice, k reordered to match hxT; chunk layout
        # [p, c=a*2+half, jj, q] with lhsT(a,j)[p,q] = W[a*128+q, j*128+p]
        Wf = w_f[hsl]
        Wfr = np.concatenate([Wf[:, hsl], Wf[:, psl], Wf[:, H:]], axis=1)
        wf = np.ascontiguousarray(
            Wfr.reshape(NHT, 128, NKT, 128).transpose(3, 0, 2, 1)
            .reshape(128, 16, 16, 128).astype(BF))
        # w_n natural k order (fgx domain is global), [p, a, j, q]
        wn = np.ascontiguousarray(
            w_n[hsl].reshape(NHT, 128, NKT, 128).transpose(3, 0, 2, 1)
            .astype(BF))

        bfp = (b_f[hsl] + 1.0) * 0.5
        in_maps.append({
            "hxT": hxT,
            "inpT": inpT,
            "wf": wf,
            "wn": wn,
            "bfp": np.ascontiguousarray(bfp.reshape(NHT, 128).T, dtype=np.float32),
            "bfm": np.ascontiguousarray((1.0 - bfp).reshape(NHT, 128).T, dtype=np.float32),
            "bn": np.ascontiguousarray(b_n[hsl].reshape(NHT, 128).T, dtype=np.float32),
        })
    return in_maps


def assemble(results):
    """[NHT,128,BL] bf16 hy^T per core -> full [B, H] f32."""
    out = np.empty((B, H), dtype=np.float32)
    for core in range(R * C):
        g, hc = core // C, core % C
        hyT = np.asarray(results[core]["hy"], dtype=np.float32)
        out[g * BL:(g + 1) * BL, hc * HC:(hc + 1) * HC] = \
            hyT.reshape(HC, BL).T
    return out


def kernel(input, hx, w_f, b_f, w_n, b_n, **_ignored):
    nc = _get_nc()
    in_maps = make_in_maps(input, hx, w_f, b_f, w_n, b_n)
    res = run_bass_kernel_spmd(nc, in_maps, list(range(R * C)))
    return assemble(res.results)


if __name__ == "__main__":
    rng = np.random.default_rng(0)
    inputs = {
        "input": rng.uniform(-1, 1, (B, I)).astype(np.float32),
        "hx": rng.uniform(-1, 1, (B, H)).astype(np.float32),
        "w_f": (rng.standard_normal((H, H + I)) / np.sqrt(H + I)).astype(np.float32),
        "b_f": (rng.standard_normal(H) / np.sqrt(H + I)).astype(np.float32),
        "w_n": (rng.standard_normal((H, H + I)) / np.sqrt(H + I)).astype(np.float32),
        "b_n": (rng.standard_normal(H) / np.sqrt(H + I)).astype(np.float32),
    }
    out = kernel(**inputs)
    x64 = {k: v.astype(np.float64) for k, v in inputs.items()}
    cat = np.concatenate([x64["hx"], x64["input"]], axis=1)
    fg = (cat @ x64["w_f"].T + x64["b_f"] + 1.0) * 0.5
    fgx = fg * x64["hx"]
    ng = np.concatenate([fgx, x64["input"]], axis=1) @ x64["w_n"].T + x64["b_n"]
    exp = (1.0 - fg) * ng + fgx
    err = np.abs(out - exp).max() / np.abs(exp).max()
    print("rel err:", err)
